# revision 1
# baseline (speedup 1.0000x reference)
"""GridMask apply (BatchHide): out = feature * mask, mask broadcast over channels.

feature: [32, 128, 224, 224] f32, mask: [32, 1, 224, 224] f32.
Data-parallel over batch across 8 NeuronCores (4 samples per core).

Per-core layout: flatten H*W = 50176 = 128 * 392 and put the 128-chunk of
spatial positions on SBUF partitions, channels on the free dim. The mask tile
[128, 392] then has exactly the same partition mapping as every channel's
feature tile, so it is loaded once per sample and reused across all 128
channels via a free-dim (stride-0) broadcast AP — zero broadcast traffic.
"""

import numpy as np

import concourse.bacc as bacc
import concourse.tile as tile
from concourse import mybir
from concourse.bass_utils import run_bass_kernel_spmd

B, C, H, W = 32, 128, 224, 224
N_CORES = 8
B_LOC = B // N_CORES  # 4 samples per core
HW = H * W  # 50176
P = 128
F = HW // P  # 392
F32 = mybir.dt.float32

_nc_cache = {}


def _build(g=128, ct=16, bufs=6, dual_ring=True):
    """g: hw-groups per tile (partition dim = (128//g channel-reps) x g hw-groups).
    Contiguous DRAM run per partition = (HW//g)*4 bytes. ct: channels per tile.
    """
    cpg = P // g  # channels covered by the partition dim
    m = ct // cpg  # channel repeats along the free dim
    t = HW // g  # hw elems per partition chunk
    assert cpg * m == ct and g * t == HW and C % ct == 0

    nc = bacc.Bacc("TRN2", target_bir_lowering=False, debug=False, num_devices=N_CORES)
    feat = nc.dram_tensor("feature", [B_LOC, C, HW], F32, kind="ExternalInput").ap()
    msk = nc.dram_tensor("mask", [B_LOC, HW], F32, kind="ExternalInput").ap()
    out = nc.dram_tensor("out", [B_LOC, C, HW], F32, kind="ExternalOutput").ap()

    # Channel-tile widths per batch: taper the first tiles of batch 0 (start
    # compute sooner) and the last tiles of the final batch (shorter drain).
    def widths(b):
        w = [ct] * (C // ct)
        rest = [ct - 8] if ct > 8 else []
        if cpg == 1 and b == 0 and ct >= 8:
            w = [4, 4] + rest + w[1:]
        if cpg == 1 and b == B_LOC - 1 and ct >= 8:
            w = w[:-1] + rest + [4, 2, 2]
        assert sum(w) == C
        return w

    with tile.TileContext(nc) as tc:
        with (
            tc.tile_pool(name="mask", bufs=B_LOC) as mpool,
            tc.tile_pool(name="data", bufs=bufs) as dpool,
        ):
            # All masks upfront on the (initially idle) scalar ring.
            mts = []
            for b in range(B_LOC):
                mt = mpool.tile([P, t], F32)
                mbc = msk[b].rearrange("(g t) -> g t", g=g)[None, :, :].broadcast_to(
                    [cpg, g, t]
                )
                nc.scalar.dma_start(out=mt[:], in_=mbc)
                mts.append(mt)
            it = 0
            for b in range(B_LOC):
                mt = mts[b]
                for w, c0 in zip(widths(b), np.cumsum([0] + widths(b)[:-1])):
                    c0 = int(c0)
                    mi = w // cpg  # channel repeats along free dim for this tile
                    fv = feat[b, c0 : c0 + w].rearrange(
                        "(m cg) (g t) -> (cg g) m t", cg=cpg, g=g
                    )
                    ov = out[b, c0 : c0 + w].rearrange(
                        "(m cg) (g t) -> (cg g) m t", cg=cpg, g=g
                    )
                    if dual_ring and it % 2 == 1:
                        ld, st = nc.scalar, nc.sync
                    else:
                        ld, st = nc.sync, nc.scalar
                    it += 1
                    ft = dpool.tile([P, m, t], F32, tag="data")
                    nc_ft = ft[:, :mi, :]
                    ld.dma_start(out=nc_ft, in_=fv)
                    nc.vector.tensor_mul(
                        out=nc_ft,
                        in0=nc_ft,
                        in1=mt[:, None, :].broadcast_to([P, mi, t]),
                    )
                    st.dma_start(out=ov, in_=nc_ft)
    nc.compile()
    return nc


def _get_nc(**kw):
    key = tuple(sorted(kw.items()))
    if key not in _nc_cache:
        _nc_cache[key] = _build(**kw)
    return _nc_cache[key]


def kernel(feature, mask):
    feature = np.ascontiguousarray(np.asarray(feature, dtype=np.float32))
    mask = np.ascontiguousarray(np.asarray(mask, dtype=np.float32))
    nc = _get_nc()
    in_maps = [
        {
            "feature": feature[i * B_LOC : (i + 1) * B_LOC].reshape(B_LOC, C, HW),
            "mask": mask[i * B_LOC : (i + 1) * B_LOC].reshape(B_LOC, HW),
        }
        for i in range(N_CORES)
    ]
    res = run_bass_kernel_spmd(nc, in_maps, list(range(N_CORES))).results
    return np.concatenate(
        [res[i]["out"].reshape(B_LOC, C, H, W) for i in range(N_CORES)], axis=0
    )

